# revision 32
# baseline (speedup 1.0000x reference)
"""GroupQueryAttention on 8 trn2 cores (bf16 compute, fp32 accumulate).

Sharding: core c = (b, g) with b = c // 4 (batch), g = c % 4 (KV group).
Each core computes the 4 query heads of its group against its batch's
sequence plus the row-slice of the output projection for those heads.
Host sums the 4 partial outputs per batch (row-parallel Wo) and adds bo.

All matmul operands are bf16 (psum accumulation fp32): same PE streaming
rate as fp32r at N=512 but none of fp32r's pre-rounding/base-partition
constraints, and half the host->device transfer bytes.  x arrives
pre-transposed ([E, S]) and weights pre-tiled ([128, chunk, cols]) so
every input DMA is a fully-contiguous burst and the PE does no transposes
except the small V retile.

Per-core schedule:
  xT   [e=128 x 8, s=2048] bf16   DMA'd directly (host transposed)
  qT   [d=64, h=4, s=2048] bf16   = Wq_g^T x^T + bq, M=128 packed matmuls
  kvT  [128, s=2048] bf16         rows 0:64 k^T, 64:128 v^T (+bkv)
  v_aug[t=128 x 16, 65] bf16      v re-transposed (PE), col 64 = 1.0
  per (s-half, head): for t in 16 tiles:
      scoresT psum [t=128, 1024] = k^T(tile)^T @ q^T   (2 matmuls)
      E = exp(0.125 * scoresT) -> bf16                 (1 ACT op per tile)
      U^T psum [65, 1024] += v_aug(t)^T @ E            (row 64 = Z)
    The A@V matmuls for tile t are emitted after the scores matmuls for
    tile t+3 (DEPTH=3 software pipeline) so exp(t) runs on ACT strictly
    under the PE's scores stream; phase 3 is ACT-exp-bound (~1.04us/tile).
  normalize (no PE): U|Z -> SBUF, 1/Z at partition 0 (DVE), broadcast to
    64 partitions with gpsimd partition_broadcast (Pool), one fused DVE
    multiply writes ubT pairs (odd heads partition-shifted to 64:128)
  out psum [s=128, e=1024] = (ubT pair)^T @ Wo rows, K=128 packed
      -> bf16 (DVE/ACT alternating) -> DMA to ot[S, E]
  host: out[b] = sum_g ot_g + bo
"""

import numpy as np
from contextlib import ExitStack

import ml_dtypes

import concourse.bass as bass
import concourse.bacc as bacc
import concourse.mybir as mybir
from concourse.tile import TileContext
from concourse.bass_utils import run_bass_kernel_spmd
B, S, E = 2, 2048, 1024
H, G, HD = 16, 4, 64
GH = H // G          # heads per group = 4
DG = GH * HD         # q cols per group = 256
N_CORES = 8

FP = mybir.dt.float32
BF = mybir.dt.bfloat16
BF_NP = ml_dtypes.bfloat16

KE = E // 128        # 8 contraction chunks for projections
NT = S // 128        # 16 t tiles
SC = 512             # matmul moving-dim chunk
NSC = S // SC        # 4
SH = 1024            # s-half for attention psum accumulators
NSH = S // SH        # 2


def build_program(loop_n: int = 1, upto: int = 4) -> bass.Bass:
    # Bacc (not plain Bass): its compile() runs move_matmul_waits_to_ldweights
    # + generate_event_semaphores, without which walrus rejects matmuls that
    # accumulated >1 semaphore wait ("Too many sync wait commands").
    nc = bacc.Bacc(None, target_bir_lowering=False)
    xt = nc.dram_tensor("xt", [E, S], BF, kind="ExternalInput")
    wq = nc.dram_tensor("wq", [128, KE, DG], BF, kind="ExternalInput")
    wkv = nc.dram_tensor("wkv", [128, KE, 2 * HD], BF, kind="ExternalInput")
    wo = nc.dram_tensor("wo", [128, DG // 128, E], BF, kind="ExternalInput")
    bq = nc.dram_tensor("bq", [DG], FP, kind="ExternalInput")
    bkv = nc.dram_tensor("bkv", [2 * HD], FP, kind="ExternalInput")
    ot = nc.dram_tensor("ot", [S, E], BF, kind="ExternalOutput")

    with TileContext(nc) as tc, ExitStack() as ctx:
        const = ctx.enter_context(tc.tile_pool(name="const", bufs=1))
        big = ctx.enter_context(tc.tile_pool(name="big", bufs=1))
        zpool = ctx.enter_context(tc.tile_pool(name="zpool", bufs=2))
        outp = ctx.enter_context(tc.tile_pool(name="outp", bufs=3))
        # PSUM banks: psc 3x2 + pav 2 = 8 of 8
        pscp = ctx.enter_context(tc.tile_pool(name="pscp", bufs=3, space="PSUM"))
        pavp = ctx.enter_context(tc.tile_pool(name="pavp", bufs=1, space="PSUM"))

        # ---- constants (outside any repeat loop) ----
        ones_bf = const.tile([128, 128], BF)
        nc.vector.memset(ones_bf, 1.0)
        bv_row = const.tile([1, HD], BF)

        wq_sb = const.tile([128, KE, DG], BF)
        wkv_sb = const.tile([128, KE, 2 * HD], BF)
        wo_sb = const.tile([128, DG // 128, E], BF)
        bq_sb = const.tile([128, DG // 128], FP)
        bkv_sb = const.tile([128, 1], FP)

        # ---- persistent activations ----
        xT = big.tile([128, NSC, KE, SC], BF)     # 32 KB/part, chunk-major
        qT = big.tile([128, DG // 128, S], BF)    # head pairs stacked
        kvT = big.tile([128, S], BF)              # 4 KB
        v_aug = big.tile([128, NT, HD + 1], BF)   # v | ones
        esb_ring = big.tile([128, 6, SH], BF)     # manual exp-output ring
        ubT = big.tile([128, DG // 128, S], BF)   # head pairs stacked

        def emit_body():
            xt_r = xt.rearrange("(j p) s -> p j s", p=128)

            # PE pstate warmup: dep-free tiny matmuls keep the tensor engine
            # continuously busy through the initial DMA window so the first
            # projection matmuls run at full clock (ramp needs ~3us busy).
            if upto >= 2:
                wup = pscp.tile([128, SH], FP, tag="psc")
                for _ in range(250):
                    nc.tensor.matmul(
                        wup[0:NT, 0:NT],
                        ones_bf[:, 0:NT],
                        ones_bf[:, 0:NT],
                        start=True,
                        stop=True,
                    )
                # dummy reader keeps the verifier happy; ubT is fully
                # overwritten by the normalize muls before phase 4 reads it
                nc.vector.tensor_copy(out=ubT[0:NT, 0, 0:NT], in_=wup[0:NT, 0:NT])

            # ---- phases 1+2 interleaved per 512-wide s-chunk:
            # DMA x^T chunk, then project it (q packed M=128: 2 heads/matmul)
            for sc in range(NSC):
                ssl = bass.ts(sc, SC)
                nc.sync.dma_start(out=xT[:, sc, :, :], in_=xt_r[:, :, ssl])
                if sc == 0:
                    nc.sync.dma_start(out=wq_sb, in_=wq[:, :, :])
                    nc.sync.dma_start(
                        out=bq_sb, in_=bq.rearrange("(j p) -> p j", p=128)
                    )
                    nc.sync.dma_start(out=wkv_sb, in_=wkv[:, :, :])
                    nc.sync.dma_start(
                        out=bkv_sb, in_=bkv.rearrange("(j p) -> p j", p=128)
                    )
                    bvt = zpool.tile([1, HD], FP, tag="bvt")
                    nc.sync.dma_start(
                        out=bvt, in_=bkv.rearrange("(j d) -> j d", j=2)[1:2, :]
                    )
                    nc.vector.tensor_copy(out=bv_row, in_=bvt)
                elif sc == 1:
                    nc.sync.dma_start(out=wo_sb, in_=wo[:, :, :])
                if upto < 2:
                    continue
                for m in range(DG // 128):
                    pq = pscp.tile([128, SH], FP, tag="psc")
                    for j in range(KE):
                        nc.tensor.matmul(
                            pq[:, 0:SC],
                            wq_sb[:, j, bass.ts(m, 128)],
                            xT[:, sc, j, :],
                            start=(j == 0),
                            stop=(j == KE - 1),
                        )
                    if m == 0:
                        nc.vector.tensor_scalar_add(
                            out=qT[:, m, ssl],
                            in0=pq[:, 0:SC],
                            scalar1=bq_sb[:, m : m + 1],
                        )
                    else:
                        nc.scalar.add(
                            out=qT[:, m, ssl], in_=pq[:, 0:SC],
                            add=bq_sb[:, m : m + 1],
                        )
                pkv = pscp.tile([128, SH], FP, tag="psc")
                for j in range(KE):
                    nc.tensor.matmul(
                        pkv[0:HD, 0:SC],
                        wkv_sb[:, j, 0:HD],
                        xT[:, sc, j, :],
                        start=(j == 0),
                        stop=(j == KE - 1),
                    )
                # k^T duplicated to both partition halves so scores can use
                # matching partition offsets for odd heads (qT pair layout)
                nc.vector.tensor_scalar_add(
                    out=kvT[0:HD, ssl],
                    in0=pkv[0:HD, 0:SC],
                    scalar1=bkv_sb[0:HD, 0:1],
                )
                nc.scalar.add(
                    out=kvT[HD : 2 * HD, ssl], in_=pkv[0:HD, 0:SC],
                    add=bkv_sb[0:HD, 0:1],
                )
                # v directly in [s, d] layout for the A@V stationary:
                # out[s, d] = x^T(chunk)^T @ Wv + 1 (x) bv, 4 chunks per sc
                pv = pscp.tile([128, SH], FP, tag="psc")
                for cc in range(SC // 128):
                    ci = sc * (SC // 128) + cc
                    for j in range(KE):
                        nc.tensor.matmul(
                            pv[:, bass.ds(cc * HD, HD)],
                            xT[:, sc, j, bass.ts(cc, 128)],
                            wkv_sb[:, j, HD : 2 * HD],
                            start=(j == 0),
                            stop=False,
                        )
                    nc.tensor.matmul(
                        pv[:, bass.ds(cc * HD, HD)],
                        ones_bf[0:1, :],
                        bv_row[:, :],
                        start=False,
                        stop=True,
                    )
                nc.vector.tensor_copy(
                    out=v_aug[:, bass.ds(sc * (SC // 128), SC // 128), 0:HD],
                    in_=pv[:, 0 : (SC // 128) * HD].rearrange(
                        "p (a b) -> p a b", b=HD
                    ),
                )

            if upto < 3:
                return

            # ones column of v_aug (Z row of the A@V accumulator)
            ones_v = ones_bf[:, 0:NT].rearrange("p (a b) -> p a b", b=1)
            nc.vector.tensor_copy(out=v_aug[:, :, HD : HD + 1], in_=ones_v)

            # ---- phase 3: attention per (s-half, head) ----
            for sh in range(NSH):
                for h in range(GH):
                    pav = pavp.tile([HD + 1, SH], FP, tag="pav")
                    esbs = {}
                    DEPTH = 3  # A@V for tile t issues after scores(t+DEPTH)
                    for t in range(NT + DEPTH):
                        if t < NT:
                            psc = pscp.tile([128, SH], FP, tag="psc")
                            po2 = (h % 2) * HD
                            for u in range(SH // SC):
                                nc.tensor.matmul(
                                    psc[:, bass.ts(u, SC)],
                                    kvT[po2 : po2 + HD, bass.ts(t, 128)],
                                    qT[po2 : po2 + HD, h // 2,
                                       bass.ds(sh * SH + u * SC, SC)],
                                    start=True,
                                    stop=True,
                                )
                            esb = esb_ring[:, t % 6, :]
                            nc.scalar.activation(
                                out=esb, in_=psc,
                                func=mybir.ActivationFunctionType.Exp,
                                scale=1.0 / np.sqrt(HD),
                            )
                            esbs[t] = esb
                        ta = t - DEPTH
                        if ta >= 0:
                            esb_a = esbs.pop(ta)
                            for u in range(SH // SC):
                                nc.tensor.matmul(
                                    pav[:, bass.ts(u, SC)],
                                    v_aug[:, ta, :],
                                    esb_a[:, bass.ts(u, SC)],
                                    start=(ta == 0),
                                    stop=(ta == NT - 1),
                                )
                    # ---- normalize (no PE): U|Z -> SBUF, bcast 1/Z, mul ----
                    ustg = zpool.tile([HD + 1, SH], FP, tag="ustg")
                    nc.vector.tensor_copy(out=ustg, in_=pav)
                    zr = zpool.tile([1, SH], FP, tag="zr")
                    nc.vector.tensor_copy(out=zr, in_=ustg[HD : HD + 1, :])
                    nc.vector.reciprocal(zr, zr)
                    zbc = zpool.tile([HD, SH], FP, tag="zbc")
                    nc.gpsimd.partition_broadcast(zbc, zr)
                    po2 = (h % 2) * HD
                    pair = h // 2
                    nc.vector.tensor_mul(
                        out=ubT[po2 : po2 + HD, pair, bass.ts(sh, SH)],
                        in0=ustg[0:HD, :],
                        in1=zbc,
                    )

            if upto < 4:
                return

            # ---- phase 4: output projection, out[s, e] ----
            for st in range(S // 128):
                ssl = bass.ts(st, 128)
                po = pscp.tile([128, SH], FP, tag="psc")
                for e2 in range(E // SC):
                    for j in range(DG // 128):
                        nc.tensor.matmul(
                            po[:, bass.ts(e2, SC)],
                            ubT[:, j, ssl],
                            wo_sb[:, j, bass.ts(e2, SC)],
                            start=(j == 0),
                            stop=(j == DG // 128 - 1),
                        )
                ost = outp.tile([128, E], BF, tag="ost")
                if st % 2 == 0:
                    nc.vector.tensor_copy(out=ost, in_=po)
                else:
                    nc.scalar.copy(out=ost, in_=po)
                nc.sync.dma_start(out=ot[ssl, :], in_=ost)

        if loop_n == 1:
            emit_body()
        else:
            with tc.For_i(0, loop_n):
                emit_body()

    nc.compile()
    return nc


_prog_cache: dict[str, bass.Bass] = {}


def _in_maps(x, Wq, bq, Wk, bk, Wv, bv, Wo, bo):
    xf = np.asarray(x, dtype=np.float32)
    Wqb = np.asarray(Wq, dtype=np.float32).astype(BF_NP)
    Wkb = np.asarray(Wk, dtype=np.float32).astype(BF_NP)
    Wvb = np.asarray(Wv, dtype=np.float32).astype(BF_NP)
    Wob = np.asarray(Wo, dtype=np.float32).astype(BF_NP)
    bqf = np.asarray(bq, dtype=np.float32)
    bkf = np.asarray(bk, dtype=np.float32)
    bvf = np.asarray(bv, dtype=np.float32)

    def tile_rows(w):
        # [n*128, c] -> [128, n, c] with row j*128+p at [p, j]
        n = w.shape[0] // 128
        return np.ascontiguousarray(w.reshape(n, 128, -1).transpose(1, 0, 2))

    maps = []
    for c in range(N_CORES):
        b, g = c // G, c % G
        wkv_g = np.concatenate(
            [Wkb[:, g * HD : (g + 1) * HD], Wvb[:, g * HD : (g + 1) * HD]], axis=1
        )
        maps.append(
            {
                "xt": np.ascontiguousarray(xf[b].T).astype(BF_NP),
                "wq": tile_rows(Wqb[:, g * DG : (g + 1) * DG]),
                "wkv": tile_rows(wkv_g),
                "wo": tile_rows(Wob[g * DG : (g + 1) * DG, :]),
                "bq": np.ascontiguousarray(bqf[g * DG : (g + 1) * DG]),
                "bkv": np.ascontiguousarray(
                    np.concatenate(
                        [bkf[g * HD : (g + 1) * HD], bvf[g * HD : (g + 1) * HD]]
                    )
                ),
            }
        )
    return maps


def kernel(x, Wq, bq, Wk, bk, Wv, bv, Wo, bo):
    if "nc" not in _prog_cache:
        _prog_cache["nc"] = build_program()
    nc = _prog_cache["nc"]

    in_maps = _in_maps(x, Wq, bq, Wk, bk, Wv, bv, Wo, bo)
    global _last_in_maps
    _last_in_maps = in_maps
    res = run_bass_kernel_spmd(nc, in_maps, list(range(N_CORES))).results

    bo = np.asarray(bo, dtype=np.float32)
    out = np.empty((B, S, E), dtype=np.float32)
    for b in range(B):
        acc = res[b * G]["ot"].astype(np.float32)
        for g in range(1, G):
            acc = acc + res[b * G + g]["ot"].astype(np.float32)
        out[b] = acc + bo
    return out
